# revision 3
# baseline (speedup 1.0000x reference)
"""Margin-based triplet loss (nn_Criterion) for Trainium2, 8 NeuronCores — v4.

v3 pipeline + symmetric pair assignment: since every row of the batch is
some core's anchor, pair (a, b) can be computed by owner(a) at
strip[chunk(b), col(a)] OR by owner(b) at strip[chunk(a), col(b)].
Circular rule: core i computes pairs whose other-end block j satisfies
(j - i) mod 8 <= 4, so each core materializes Gram strips for only the
5 blocks i..i+4 (20 chunks instead of 32). Gather, drain, matmul, and
DMA volume all drop ~40%; load stays balanced (~16.4K pairs/core).

Per local b-chunk of 128 rows the device: 4 fp8 DoubleRow matmuls + one
fp8 fold matmul (-(na+SQB)/2 in hi/lo rows) accumulate d^2 into PSUM;
ACT/DVE drain to f32 strips; GPSIMD ap_gather pulls the triplet columns
per 16-partition group (first/last quads in 2-chunk halves); ACT sqrts
the gathered tile; DVE does 4 count/sum passes against fp8 sel weights.
Row norms ship from the host (nbc local-chunk layout + fold rows).

Each core returns [P, 4*NACC] accum columns; the host sums them into
[cntP, sumP, cntN, sumN] and applies a first-order Taylor correction
for the SQB bias on the pos side.
"""

import os

import numpy as np

B, D, T, NCLS = 4096, 1024, 65536, 100
MARGIN = 0.2
NCORES = 8
P = 128
RB = 512           # anchors per core
NBLK = 5           # strip blocks per core (circular i..i+4)
NCHL = 4 * NBLK    # 20 local b-chunks
NQ = NBLK          # 5 quads
KS = 96            # slots per side per 16-partition group (max seen: 89)
K = 2 * KS         # combined slots per chunk (pos | neg)
KQ = 4 * K         # slots per quad gather
NACC = NQ + 2      # accum columns (quads 0 and 4 use two each)
SQB = 0.5          # sqrt bias; absorbs fp8 fold + f32 rounding at diag
_HALF_QUADS = (0, NQ - 1)

_COMPILED = None
_THRESH = None
LAST_RESULTS = None


def _build_nc(cpos: float, cneg: float, debug: bool = False):
    import concourse.bacc as bacc
    import concourse.mybir as mybir
    import concourse.tile as tile

    f32 = mybir.dt.float32
    fp16 = mybir.dt.float16
    fp8 = mybir.dt.float8e4
    i16 = mybir.dt.int16
    Alu = mybir.AluOpType
    Act = mybir.ActivationFunctionType
    DR = mybir.MatmulPerfMode.DoubleRow

    nc = bacc.Bacc("TRN2")

    xt_d = nc.dram_tensor("xt", [P, 4, 2, NCHL * P], fp8, kind="ExternalInput")
    xat_d = nc.dram_tensor("xat", [P, 4, 2, RB], fp8, kind="ExternalInput")
    nbc_d = nc.dram_tensor("nbc", [P, NCHL], f32, kind="ExternalInput")
    narow_d = nc.dram_tensor("narow", [2, 2, RB], fp8, kind="ExternalInput")
    idx_d = nc.dram_tensor("idx", [P, NQ, KQ // 16], i16, kind="ExternalInput")
    sel_d = nc.dram_tensor("sel", [P, NQ, 4, K], fp8, kind="ExternalInput")
    out_d = nc.dram_tensor("out", [P, 4 * NACC], f32, kind="ExternalOutput")

    with tile.TileContext(nc) as tc:
        with (
            tc.tile_pool(name="big", bufs=1) as big,
            tc.tile_pool(name="stp", bufs=4) as stp,
            tc.tile_pool(name="gat", bufs=4) as gat,
            tc.tile_pool(name="mid", bufs=4) as mid,
            tc.tile_pool(name="small", bufs=1) as small,
            tc.tile_pool(name="gpsum", bufs=8, space="PSUM") as gpsum,
        ):
            xt = big.tile([P, 4, 2, NCHL * P], fp8, tag="xt", name="xt")
            xat = big.tile([P, 4, 2, RB], fp8, tag="xat", name="xat")
            nbc = small.tile([P, NCHL], f32, tag="nbc", name="nbc")
            idxt = big.tile([P, NQ, KQ // 16], i16, tag="idx", name="idx")
            selt = big.tile([P, NQ, 4, K], fp8, tag="sel", name="sel")
            narow = small.tile([2, 2, RB], fp8, tag="narow", name="narow")
            ones4 = small.tile([2, 2, P], fp8, tag="ones4", name="ones4")
            dum = small.tile([1, 8], f32, tag="dum", name="dum")

            # Warm the ACT function table that contains Sqrt before the
            # pipeline needs it (saves a mid-stream LoadActFuncSet).
            nc.vector.memset(dum[:], 1.0)
            nc.scalar.activation(dum[:], dum[:], Act.Sqrt)
            nc.vector.memset(ones4[:], 4.0)

            # DMA order: serialized stream; lead with quad 0's operands so
            # PE warms up immediately, trail sel slices (needed latest).
            nc.sync.dma_start(xat[:, 0], xat_d[:, 0])
            nc.sync.dma_start(xt[:, 0, :, 0:512], xt_d[:, 0, :, 0:512])
            nc.sync.dma_start(narow[:], narow_d[:])
            nc.sync.dma_start(nbc[:], nbc_d[:])
            nc.sync.dma_start(idxt[:], idx_d[:])
            for t in range(1, 4):
                nc.sync.dma_start(xat[:, t], xat_d[:, t])
                nc.sync.dma_start(xt[:, t, :, 0:512], xt_d[:, t, :, 0:512])
            for blk in range(1, NBLK):
                cols = slice(512 * blk, 512 * (blk + 1))
                nc.sync.dma_start(xt[:, :, :, cols], xt_d[:, :, :, cols])
                if blk >= 2:
                    nc.sync.dma_start(selt[:, blk - 2], sel_d[:, blk - 2])
            nc.sync.dma_start(selt[:, 3], sel_d[:, 3])
            nc.sync.dma_start(selt[:, 4], sel_d[:, 4])

            acct = small.tile([P, 4 * NACC], f32, tag="acct", name="acct")

            def do_chunk(s4, ch, j):
                g = gpsum.tile([P, RB], f32, tag="g", name="g", space="PSUM")
                for t in range(4):
                    nc.tensor.matmul(
                        g[:],
                        xt[:, t, :, ch * P : (ch + 1) * P],
                        xat[:, t],
                        start=(t == 0),
                        stop=False,
                        perf_mode=DR,
                    )
                # fold -(na+SQB)/2 via fp8 hi/lo rows scaled by 4.0
                nc.tensor.matmul(
                    g[:], ones4[:], narow[:],
                    start=False, stop=True, perf_mode=DR,
                )
                # strip: s = -2*G + na + nb + SQB = d^2 + SQB
                # (ACT only: DVE tensor_scalar drains showed a rare
                # nondeterministic race against the Pool gather)
                nc.scalar.activation(
                    s4[:, j], g[:], Act.Identity,
                    bias=nbc[:, ch : ch + 1], scale=-2.0,
                )

            def chain(q, gth, c0, c1, acol):
                # d = sqrt(gathered); masks + accumulation over chunks c0:c1
                nck = c1 - c0
                dg = mid.tile([P, nck, K], fp16, tag="dg", name="dg")
                nc.scalar.activation(dg[:], gth[:, c0 * K : c1 * K], Act.Sqrt)
                Am = mid.tile([P, nck, K], fp16, tag="Am", name="Am")
                jk = mid.tile([P, nck, K], fp16, tag="jk", name="jk")
                sl = selt[:, q, c0:c1]
                nc.vector.scalar_tensor_tensor(
                    Am[:, :, 0:KS], dg[:, :, 0:KS], cpos, sl[:, :, 0:KS],
                    Alu.is_gt, Alu.mult,
                    accum_out=acct[:, 0 * NACC + acol : 0 * NACC + acol + 1],
                )
                nc.vector.scalar_tensor_tensor(
                    jk[:, :, 0:KS], dg[:, :, 0:KS], 1.0, Am[:, :, 0:KS],
                    Alu.mult, Alu.mult,
                    accum_out=acct[:, 1 * NACC + acol : 1 * NACC + acol + 1],
                )
                nc.vector.scalar_tensor_tensor(
                    Am[:, :, KS:K], dg[:, :, KS:K], cneg, sl[:, :, KS:K],
                    Alu.is_lt, Alu.mult,
                    accum_out=acct[:, 2 * NACC + acol : 2 * NACC + acol + 1],
                )
                nc.vector.scalar_tensor_tensor(
                    jk[:, :, KS:K], dg[:, :, KS:K], 1.0, Am[:, :, KS:K],
                    Alu.mult, Alu.mult,
                    accum_out=acct[:, 3 * NACC + acol : 3 * NACC + acol + 1],
                )

            jobs = []

            def flush(keep):
                while len(jobs) > keep:
                    chain(*jobs.pop(0))

            acol_half = {0: (0, NQ), NQ - 1: (NQ - 1, NQ + 1)}
            for q in range(NQ):
                s4 = stp.tile([P, 4, RB], f32, tag="s4", name="s4")
                if q in _HALF_QUADS:
                    ca, cb = acol_half[q]
                    gth = gat.tile([P, KQ], f32, tag="gth", name="gth")
                    for j in range(2):
                        do_chunk(s4, 4 * q + j, j)
                    flush(1)
                    nc.gpsimd.ap_gather(
                        gth[:, 0 : 2 * K], s4[:, 0:2], idxt[:, q, 0 : KQ // 32],
                        channels=P, num_elems=2 * RB, d=1, num_idxs=2 * K,
                    )
                    jobs.append((q, gth, 0, 2, ca))
                    for j in range(2, 4):
                        do_chunk(s4, 4 * q + j, j)
                    flush(1)
                    nc.gpsimd.ap_gather(
                        gth[:, 2 * K : KQ], s4[:, 2:4],
                        idxt[:, q, KQ // 32 : KQ // 16],
                        channels=P, num_elems=2 * RB, d=1, num_idxs=2 * K,
                    )
                    jobs.append((q, gth, 2, 4, cb))
                else:
                    for j in range(4):
                        do_chunk(s4, 4 * q + j, j)
                    gth = gat.tile([P, KQ], f32, tag="gth", name="gth")
                    flush(1)
                    nc.gpsimd.ap_gather(
                        gth[:], s4[:], idxt[:, q],
                        channels=P, num_elems=4 * RB, d=1, num_idxs=KQ,
                    )
                    jobs.append((q, gth, 0, 4, q))
            flush(0)

            nc.sync.dma_start(out_d[:], acct[:])

    nc.compile()
    return nc


def _cumcount(key):
    """Per-element occurrence index within its key group (vectorized)."""
    n = len(key)
    if n == 0:
        return np.zeros(0, dtype=np.int64)
    order = np.argsort(key, kind="stable")
    sk = key[order]
    start = np.r_[0, np.flatnonzero(sk[1:] != sk[:-1]) + 1]
    lens = np.diff(np.r_[start, n])
    pos = np.arange(n) - np.repeat(start, lens)
    out = np.empty(n, dtype=np.int64)
    out[order] = pos
    return out


def _prep_inputs(batch, labels, triplets, beta):
    import ml_dtypes

    fp8 = ml_dtypes.float8_e4m3

    bv = np.asarray(beta, dtype=np.float64)
    assert np.allclose(bv, bv[0]), "kernel assumes per-class beta is constant"

    trip = np.asarray(triplets).astype(np.int64)
    xq = np.clip(np.asarray(batch, dtype=np.float32), -240.0, 240.0).astype(fp8)
    xqf = xq.astype(np.float32)

    xt_glob = np.ascontiguousarray(
        xqf.T.reshape(4, 2, P, B).transpose(2, 0, 1, 3)
    ).astype(fp8)                                       # [P, 4, 2, B]
    na = (xqf.astype(np.float64) ** 2).sum(1)          # [B] row norms, f64
    na2 = na.reshape(NCORES * 4, P)                    # per global chunk

    # symmetric assignment: pair (a, b) computed by owner(a) if
    # (owner(b) - owner(a)) mod 8 <= 4, else by owner(b); strips at the
    # computing core cover blocks i..i+4 (local chunk = position there)
    comp_l, ch_l, part_l, grp_l, col_l, slot_l, side_l = [], [], [], [], [], [], []
    for side, col in ((0, 1), (1, 2)):
        a = trip[:, 0]
        b = trip[:, col]
        i = a >> 9
        j = b >> 9
        comp = np.where((j - i) % 8 <= 4, i, j)
        row = np.where(comp == i, b, a)
        colm = np.where(comp == i, a, b)
        chg = row >> 7
        ch = 4 * (((chg >> 2) - comp) % 8) + (chg & 3)   # local chunk 0..19
        part = row & 127
        grp = part >> 4
        k = _cumcount(((comp * NCHL + ch) * 8 + grp))
        kmax = int(k.max()) if len(k) else 0
        assert kmax < KS, f"slot overflow side {side}: {kmax}"
        comp_l.append(comp); ch_l.append(ch); part_l.append(part)
        grp_l.append(grp); col_l.append(colm - 512 * comp)
        slot_l.append(side * KS + k); side_l.append(side)

    in_maps = []
    for c in range(NCORES):
        lo = c * RB
        xa = xq[lo : lo + RB]
        xat_host = np.ascontiguousarray(
            xa.astype(np.float32).T.reshape(4, 2, P, RB).transpose(2, 0, 1, 3)
        ).astype(fp8)

        gchunks = [4 * ((c + q) % 8) + r for q in range(NBLK) for r in range(4)]
        xt_host = np.ascontiguousarray(np.concatenate(
            [xt_glob[:, :, :, 128 * g : 128 * (g + 1)] for g in gchunks], axis=3
        ))
        nbc_host = np.ascontiguousarray(na2[gchunks].T.astype(np.float32))

        v = -(na[lo : lo + RB] + SQB) / 8.0     # ones are 4.0; fold scale -2
        narow_host = np.zeros((2, 2, RB), dtype=fp8)
        r = v
        for rowi in range(4):
            q8 = r.astype(fp8)
            narow_host[rowi // 2, rowi % 2] = q8
            r = r - q8.astype(np.float64)

        idx = np.zeros((P, NQ, KQ // 16), dtype=np.int16)
        sel = np.zeros((P, NQ, 4, K), dtype=fp8)
        for comp, ch, part, grp, colm, slot in zip(
            comp_l, ch_l, part_l, grp_l, col_l, slot_l
        ):
            m = comp == c
            chm = ch[m]; partm = part[m]; grpm = grp[m]
            colmm = colm[m]; slotm = slot[m]
            quad = chm >> 2
            halfq = (quad == 0) | (quad == NQ - 1)
            # half quads: two 2-chunk gathers; mids: one 4-chunk gather
            i_ = np.where(halfq, K * (chm & 1) + slotm,
                          K * (chm & 3) + slotm)
            val = np.where(halfq, colmm + 512 * (chm & 1),
                           colmm + 512 * (chm & 3)).astype(np.int16)
            colbase = np.where(halfq & ((chm & 2) == 2), KQ // 32, 0)
            idx[16 * grpm + (i_ % 16), quad, colbase + i_ // 16] = val
            sel[partm, quad, chm & 3, slotm] = 1.0

        in_maps.append(
            {
                "xt": xt_host,
                "xat": xat_host,
                "nbc": nbc_host,
                "narow": narow_host,
                "idx": idx,
                "sel": sel,
            }
        )
    return in_maps


def kernel(batch, labels, triplets, beta):
    global _COMPILED, _THRESH, LAST_RESULTS
    from concourse.bass_utils import run_bass_kernel_spmd

    bv = np.asarray(beta, dtype=np.float64)
    cpos = float(bv[0]) - MARGIN
    cneg = float(bv[0]) + MARGIN
    if _COMPILED is None or _THRESH != (cpos, cneg):
        _COMPILED = _build_nc(cpos, cneg)
        _THRESH = (cpos, cneg)
    nc = _COMPILED

    in_maps = _prep_inputs(batch, labels, triplets, beta)
    trace = bool(int(os.environ.get("KERNEL_TRACE", "0")))
    res = run_bass_kernel_spmd(
        nc, in_maps, core_ids=list(range(NCORES)), trace=trace
    )
    LAST_RESULTS = res

    cntP = sumP = cntN = sumN = 0.0
    for r in res.results:
        o = r["out"].astype(np.float64).reshape(P, 4, NACC)
        cntP += o[:, 0].sum()
        sumP += o[:, 1].sum()
        cntN += o[:, 2].sum()
        sumN += o[:, 3].sum()
    pos = sumP - cpos * cntP
    if cntP > 0:
        pos -= SQB * cntP / (2.0 * (sumP / cntP))  # SQB Taylor correction
    neg = cneg * cntN - sumN
    total = pos + neg
    cnt = cntP + cntN
    loss = total if cnt == 0.0 else total / cnt
    return np.float32(loss)


# revision 4
# speedup vs baseline: 1.0579x; 1.0579x over previous
"""Margin-based triplet loss (nn_Criterion) for Trainium2, 8 NeuronCores — v4.

v3 pipeline + symmetric pair assignment: since every row of the batch is
some core's anchor, pair (a, b) can be computed by owner(a) at
strip[chunk(b), col(a)] OR by owner(b) at strip[chunk(a), col(b)].
Circular rule: core i computes pairs whose other-end block j satisfies
(j - i) mod 8 <= 4, so each core materializes Gram strips for only the
5 blocks i..i+4 (20 chunks instead of 32). Gather, drain, matmul, and
DMA volume all drop ~40%; load stays balanced (~16.4K pairs/core).

Per local b-chunk of 128 rows the device: 4 fp8 DoubleRow matmuls + one
fp8 fold matmul (-(na+SQB)/2 in hi/lo rows) accumulate d^2 into PSUM;
ACT/DVE drain to f32 strips; GPSIMD ap_gather pulls the triplet columns
per 16-partition group (first/last quads in 2-chunk halves); ACT sqrts
the gathered tile; DVE does 4 count/sum passes against fp8 sel weights.
Row norms ship from the host (nbc local-chunk layout + fold rows).

Each core returns [P, 4*NACC] accum columns; the host sums them into
[cntP, sumP, cntN, sumN] and applies a first-order Taylor correction
for the SQB bias on the pos side.
"""

import os

import numpy as np

B, D, T, NCLS = 4096, 1024, 65536, 100
MARGIN = 0.2
NCORES = 8
P = 128
RB = 512           # anchors per core
NBLK = 5           # strip blocks per core (circular i..i+4)
NCHL = 4 * NBLK    # 20 local b-chunks
NQ = NBLK          # 5 quads
KS = 96            # slots per side per 16-partition group (max seen: 89)
K = 2 * KS         # combined slots per chunk (pos | neg)
KQ = 4 * K         # slots per quad gather
NACC = NQ + 2      # accum columns (quads 0 and 4 use two each)
SQB = 0.5          # sqrt bias; absorbs fp8 fold + f32 rounding at diag
_HALF_QUADS = (0, NQ - 1)

_COMPILED = None
_THRESH = None
LAST_RESULTS = None


def _build_nc(cpos: float, cneg: float, debug: bool = False):
    import concourse.bacc as bacc
    import concourse.mybir as mybir
    import concourse.tile as tile

    f32 = mybir.dt.float32
    fp16 = mybir.dt.float16
    fp8 = mybir.dt.float8e4
    i16 = mybir.dt.int16
    Alu = mybir.AluOpType
    Act = mybir.ActivationFunctionType
    DR = mybir.MatmulPerfMode.DoubleRow

    nc = bacc.Bacc("TRN2")

    xt_d = nc.dram_tensor("xt", [P, 4, 2, NCHL * P], fp8, kind="ExternalInput")
    xat_d = nc.dram_tensor("xat", [P, 4, 2, RB], fp8, kind="ExternalInput")
    nbc_d = nc.dram_tensor("nbc", [P, NCHL], f32, kind="ExternalInput")
    narow_d = nc.dram_tensor("narow", [2, 2, RB], fp8, kind="ExternalInput")
    idx_d = nc.dram_tensor("idx", [P, NQ, KQ // 16], i16, kind="ExternalInput")
    sel_d = nc.dram_tensor("sel", [P, NQ, 4, K], fp8, kind="ExternalInput")
    out_d = nc.dram_tensor("out", [P, 4 * NACC], f32, kind="ExternalOutput")

    with tile.TileContext(nc) as tc:
        with (
            tc.tile_pool(name="big", bufs=1) as big,
            tc.tile_pool(name="stp", bufs=4) as stp,
            tc.tile_pool(name="gat", bufs=4) as gat,
            tc.tile_pool(name="mid", bufs=4) as mid,
            tc.tile_pool(name="small", bufs=1) as small,
            tc.tile_pool(name="gpsum", bufs=8, space="PSUM") as gpsum,
        ):
            xt = big.tile([P, 4, 2, NCHL * P], fp8, tag="xt", name="xt")
            xat = big.tile([P, 4, 2, RB], fp8, tag="xat", name="xat")
            nbc = small.tile([P, NCHL], f32, tag="nbc", name="nbc")
            idxt = big.tile([P, NQ, KQ // 16], i16, tag="idx", name="idx")
            selt = big.tile([P, NQ, 4, K], fp8, tag="sel", name="sel")
            narow = small.tile([2, 2, RB], fp8, tag="narow", name="narow")
            ones4 = small.tile([2, 2, P], fp8, tag="ones4", name="ones4")
            dum = small.tile([1, 8], f32, tag="dum", name="dum")

            # Warm the ACT function table that contains Sqrt before the
            # pipeline needs it (saves a mid-stream LoadActFuncSet).
            nc.vector.memset(dum[:], 1.0)
            nc.scalar.activation(dum[:], dum[:], Act.Sqrt)
            nc.vector.memset(ones4[:], 4.0)

            # DMA order: serialized stream; lead with quad 0's operands so
            # PE warms up immediately, trail sel slices (needed latest).
            for t in range(4):
                nc.sync.dma_start(xat[:, t], xat_d[:, t])
                nc.sync.dma_start(xt[:, t, :, 0:512], xt_d[:, t, :, 0:512])
            nc.sync.dma_start(narow[:], narow_d[:])
            nc.sync.dma_start(nbc[:], nbc_d[:])
            nc.sync.dma_start(idxt[:], idx_d[:])
            for blk in range(1, NBLK):
                cols = slice(512 * blk, 512 * (blk + 1))
                nc.sync.dma_start(xt[:, :, :, cols], xt_d[:, :, :, cols])
                if blk >= 2:
                    nc.sync.dma_start(selt[:, blk - 2], sel_d[:, blk - 2])
            nc.sync.dma_start(selt[:, 3], sel_d[:, 3])
            nc.sync.dma_start(selt[:, 4], sel_d[:, 4])

            acct = small.tile([P, 4 * NACC], f32, tag="acct", name="acct")

            def do_chunk(s4, ch, j):
                g = gpsum.tile([P, RB], f32, tag="g", name="g", space="PSUM")
                for t in range(4):
                    nc.tensor.matmul(
                        g[:],
                        xt[:, t, :, ch * P : (ch + 1) * P],
                        xat[:, t],
                        start=(t == 0),
                        stop=False,
                        perf_mode=DR,
                    )
                # fold -(na+SQB)/2 via fp8 hi/lo rows scaled by 4.0
                nc.tensor.matmul(
                    g[:], ones4[:], narow[:],
                    start=False, stop=True, perf_mode=DR,
                )
                # strip: s = -2*G + na + nb + SQB = d^2 + SQB
                # (ACT only: DVE tensor_scalar drains showed a rare
                # nondeterministic race against the Pool gather)
                nc.scalar.activation(
                    s4[:, j], g[:], Act.Sqrt,
                    bias=nbc[:, ch : ch + 1], scale=-2.0,
                )

            def chain(q, gth, c0, c1, acol):
                # masks + accumulation over chunks c0:c1 (strip already = d)
                nck = c1 - c0
                dg = gth[:, c0:c1]
                Am = mid.tile([P, nck, K], fp16, tag="Am", name="Am")
                jk = mid.tile([P, nck, K], fp16, tag="jk", name="jk")
                sl = selt[:, q, c0:c1]
                nc.vector.scalar_tensor_tensor(
                    Am[:, :, 0:KS], dg[:, :, 0:KS], cpos, sl[:, :, 0:KS],
                    Alu.is_gt, Alu.mult,
                    accum_out=acct[:, 0 * NACC + acol : 0 * NACC + acol + 1],
                )
                nc.vector.scalar_tensor_tensor(
                    jk[:, :, 0:KS], dg[:, :, 0:KS], 1.0, Am[:, :, 0:KS],
                    Alu.mult, Alu.mult,
                    accum_out=acct[:, 1 * NACC + acol : 1 * NACC + acol + 1],
                )
                nc.vector.scalar_tensor_tensor(
                    Am[:, :, KS:K], dg[:, :, KS:K], cneg, sl[:, :, KS:K],
                    Alu.is_lt, Alu.mult,
                    accum_out=acct[:, 2 * NACC + acol : 2 * NACC + acol + 1],
                )
                nc.vector.scalar_tensor_tensor(
                    jk[:, :, KS:K], dg[:, :, KS:K], 1.0, Am[:, :, KS:K],
                    Alu.mult, Alu.mult,
                    accum_out=acct[:, 3 * NACC + acol : 3 * NACC + acol + 1],
                )

            jobs = []

            def flush(keep):
                while len(jobs) > keep:
                    chain(*jobs.pop(0))

            acol_half = {0: (0, NQ), NQ - 1: (NQ - 1, NQ + 1)}
            for q in range(NQ):
                s4 = stp.tile([P, 4, RB], f32, tag="s4", name="s4")
                if q in _HALF_QUADS:
                    ca, cb = acol_half[q]
                    gth = gat.tile([P, 4, K], f32, tag="gth", name="gth")
                    for j in range(2):
                        do_chunk(s4, 4 * q + j, j)
                    flush(1)
                    nc.gpsimd.ap_gather(
                        gth[:, 0:2], s4[:, 0:2], idxt[:, q, 0 : KQ // 32],
                        channels=P, num_elems=2 * RB, d=1, num_idxs=2 * K,
                    )
                    jobs.append((q, gth, 0, 2, ca))
                    for j in range(2, 4):
                        do_chunk(s4, 4 * q + j, j)
                    flush(1 if q == 0 else 0)
                    nc.gpsimd.ap_gather(
                        gth[:, 2:4], s4[:, 2:4],
                        idxt[:, q, KQ // 32 : KQ // 16],
                        channels=P, num_elems=2 * RB, d=1, num_idxs=2 * K,
                    )
                    jobs.append((q, gth, 2, 4, cb))
                else:
                    for j in range(4):
                        do_chunk(s4, 4 * q + j, j)
                    gth = gat.tile([P, 4, K], f32, tag="gth", name="gth")
                    flush(1)
                    nc.gpsimd.ap_gather(
                        gth[:], s4[:], idxt[:, q],
                        channels=P, num_elems=4 * RB, d=1, num_idxs=KQ,
                    )
                    jobs.append((q, gth, 0, 4, q))
            flush(0)

            nc.sync.dma_start(out_d[:], acct[:])

    nc.compile()
    return nc


def _cumcount(key):
    """Per-element occurrence index within its key group (vectorized)."""
    n = len(key)
    if n == 0:
        return np.zeros(0, dtype=np.int64)
    order = np.argsort(key, kind="stable")
    sk = key[order]
    start = np.r_[0, np.flatnonzero(sk[1:] != sk[:-1]) + 1]
    lens = np.diff(np.r_[start, n])
    pos = np.arange(n) - np.repeat(start, lens)
    out = np.empty(n, dtype=np.int64)
    out[order] = pos
    return out


def _prep_inputs(batch, labels, triplets, beta):
    import ml_dtypes

    fp8 = ml_dtypes.float8_e4m3

    bv = np.asarray(beta, dtype=np.float64)
    assert np.allclose(bv, bv[0]), "kernel assumes per-class beta is constant"

    trip = np.asarray(triplets).astype(np.int64)
    xq = np.clip(np.asarray(batch, dtype=np.float32), -240.0, 240.0).astype(fp8)
    xqf = xq.astype(np.float32)

    xt_glob = np.ascontiguousarray(
        xqf.T.reshape(4, 2, P, B).transpose(2, 0, 1, 3)
    ).astype(fp8)                                       # [P, 4, 2, B]
    na = (xqf.astype(np.float64) ** 2).sum(1)          # [B] row norms, f64
    na2 = na.reshape(NCORES * 4, P)                    # per global chunk

    # symmetric assignment: pair (a, b) computed by owner(a) if
    # (owner(b) - owner(a)) mod 8 <= 4, else by owner(b); strips at the
    # computing core cover blocks i..i+4 (local chunk = position there)
    comp_l, ch_l, part_l, grp_l, col_l, slot_l, side_l = [], [], [], [], [], [], []
    for side, col in ((0, 1), (1, 2)):
        a = trip[:, 0]
        b = trip[:, col]
        i = a >> 9
        j = b >> 9
        comp = np.where((j - i) % 8 <= 4, i, j)
        row = np.where(comp == i, b, a)
        colm = np.where(comp == i, a, b)
        chg = row >> 7
        ch = 4 * (((chg >> 2) - comp) % 8) + (chg & 3)   # local chunk 0..19
        part = row & 127
        grp = part >> 4
        k = _cumcount(((comp * NCHL + ch) * 8 + grp))
        kmax = int(k.max()) if len(k) else 0
        assert kmax < KS, f"slot overflow side {side}: {kmax}"
        comp_l.append(comp); ch_l.append(ch); part_l.append(part)
        grp_l.append(grp); col_l.append(colm - 512 * comp)
        slot_l.append(side * KS + k); side_l.append(side)

    in_maps = []
    for c in range(NCORES):
        lo = c * RB
        xa = xq[lo : lo + RB]
        xat_host = np.ascontiguousarray(
            xa.astype(np.float32).T.reshape(4, 2, P, RB).transpose(2, 0, 1, 3)
        ).astype(fp8)

        gchunks = [4 * ((c + q) % 8) + r for q in range(NBLK) for r in range(4)]
        xt_host = np.ascontiguousarray(np.concatenate(
            [xt_glob[:, :, :, 128 * g : 128 * (g + 1)] for g in gchunks], axis=3
        ))
        nbc_host = np.ascontiguousarray(na2[gchunks].T.astype(np.float32))

        v = -(na[lo : lo + RB] + SQB) / 8.0     # ones are 4.0; fold scale -2
        narow_host = np.zeros((2, 2, RB), dtype=fp8)
        r = v
        for rowi in range(4):
            q8 = r.astype(fp8)
            narow_host[rowi // 2, rowi % 2] = q8
            r = r - q8.astype(np.float64)

        idx = np.zeros((P, NQ, KQ // 16), dtype=np.int16)
        sel = np.zeros((P, NQ, 4, K), dtype=fp8)
        for comp, ch, part, grp, colm, slot in zip(
            comp_l, ch_l, part_l, grp_l, col_l, slot_l
        ):
            m = comp == c
            chm = ch[m]; partm = part[m]; grpm = grp[m]
            colmm = colm[m]; slotm = slot[m]
            quad = chm >> 2
            halfq = (quad == 0) | (quad == NQ - 1)
            # half quads: two 2-chunk gathers; mids: one 4-chunk gather
            i_ = np.where(halfq, K * (chm & 1) + slotm,
                          K * (chm & 3) + slotm)
            val = np.where(halfq, colmm + 512 * (chm & 1),
                           colmm + 512 * (chm & 3)).astype(np.int16)
            colbase = np.where(halfq & ((chm & 2) == 2), KQ // 32, 0)
            idx[16 * grpm + (i_ % 16), quad, colbase + i_ // 16] = val
            sel[partm, quad, chm & 3, slotm] = 1.0

        in_maps.append(
            {
                "xt": xt_host,
                "xat": xat_host,
                "nbc": nbc_host,
                "narow": narow_host,
                "idx": idx,
                "sel": sel,
            }
        )
    return in_maps


def kernel(batch, labels, triplets, beta):
    global _COMPILED, _THRESH, LAST_RESULTS
    from concourse.bass_utils import run_bass_kernel_spmd

    bv = np.asarray(beta, dtype=np.float64)
    cpos = float(bv[0]) - MARGIN
    cneg = float(bv[0]) + MARGIN
    if _COMPILED is None or _THRESH != (cpos, cneg):
        _COMPILED = _build_nc(cpos, cneg)
        _THRESH = (cpos, cneg)
    nc = _COMPILED

    in_maps = _prep_inputs(batch, labels, triplets, beta)
    trace = bool(int(os.environ.get("KERNEL_TRACE", "0")))
    res = run_bass_kernel_spmd(
        nc, in_maps, core_ids=list(range(NCORES)), trace=trace
    )
    LAST_RESULTS = res

    cntP = sumP = cntN = sumN = 0.0
    for r in res.results:
        o = r["out"].astype(np.float64).reshape(P, 4, NACC)
        cntP += o[:, 0].sum()
        sumP += o[:, 1].sum()
        cntN += o[:, 2].sum()
        sumN += o[:, 3].sum()
    pos = sumP - cpos * cntP
    if cntP > 0:
        pos -= SQB * cntP / (2.0 * (sumP / cntP))  # SQB Taylor correction
    neg = cneg * cntN - sumN
    total = pos + neg
    cnt = cntP + cntN
    loss = total if cnt == 0.0 else total / cnt
    return np.float32(loss)


# revision 5
# speedup vs baseline: 1.0922x; 1.0324x over previous
"""Margin-based triplet loss (nn_Criterion) for Trainium2, 8 NeuronCores — v4.

v3 pipeline + symmetric pair assignment: since every row of the batch is
some core's anchor, pair (a, b) can be computed by owner(a) at
strip[chunk(b), col(a)] OR by owner(b) at strip[chunk(a), col(b)].
Circular rule: core i computes pairs whose other-end block j satisfies
(j - i) mod 8 <= 4, so each core materializes Gram strips for only the
5 blocks i..i+4 (20 chunks instead of 32). Gather, drain, matmul, and
DMA volume all drop ~40%; load stays balanced (~16.4K pairs/core).

Per local b-chunk of 128 rows the device: 4 fp8 DoubleRow matmuls + one
fp8 fold matmul (-(na+SQB)/2 in hi/lo rows) accumulate d^2 into PSUM;
ACT/DVE drain to f32 strips; GPSIMD ap_gather pulls the triplet columns
per 16-partition group (first/last quads in 2-chunk halves); ACT sqrts
the gathered tile; DVE does 4 count/sum passes against fp8 sel weights.
Row norms ship from the host (nbc local-chunk layout + fold rows).

Each core returns [P, 4*NACC] accum columns; the host sums them into
[cntP, sumP, cntN, sumN] and applies a first-order Taylor correction
for the SQB bias on the pos side.
"""

import os

import numpy as np

B, D, T, NCLS = 4096, 1024, 65536, 100
MARGIN = 0.2
NCORES = 8
P = 128
RB = 512           # anchors per core
NBLK = 5           # strip blocks per core (circular i..i+4)
NCHL = 4 * NBLK    # 20 local b-chunks
NQ = NBLK          # 5 quads
KS = 96            # slots per side per 16-partition group (max seen: 89)
K = 2 * KS         # combined slots per chunk (pos | neg)
KQ = 4 * K         # slots per quad gather
NACC = NQ + 2      # accum columns (quads 0 and 4 use two each)
SQB = 0.5          # sqrt bias; absorbs fp8 fold + f32 rounding at diag
_HALF_QUADS = (0, NQ - 1)

_COMPILED = None
_THRESH = None
LAST_RESULTS = None


def _build_nc(cpos: float, cneg: float, debug: bool = False):
    import concourse.bacc as bacc
    import concourse.mybir as mybir
    import concourse.tile as tile

    f32 = mybir.dt.float32
    fp16 = mybir.dt.float16
    fp8 = mybir.dt.float8e4
    i16 = mybir.dt.int16
    Alu = mybir.AluOpType
    Act = mybir.ActivationFunctionType
    DR = mybir.MatmulPerfMode.DoubleRow

    nc = bacc.Bacc("TRN2")

    xc_d = nc.dram_tensor("xc", [P, 4, 2, 2 * RB], fp8, kind="ExternalInput")
    xt_d = nc.dram_tensor("xt", [P, 4, 2, (NCHL - 4) * P], fp8, kind="ExternalInput")
    nbc_d = nc.dram_tensor("nbc", [P, NCHL], f32, kind="ExternalInput")
    narow_d = nc.dram_tensor("narow", [2, 2, RB], fp8, kind="ExternalInput")
    idx_d = nc.dram_tensor("idx", [P, NQ, KQ // 16], i16, kind="ExternalInput")
    sel_d = nc.dram_tensor("sel", [P, NQ, 4, K], fp8, kind="ExternalInput")
    out_d = nc.dram_tensor("out", [P, 4 * NACC], f32, kind="ExternalOutput")

    with tile.TileContext(nc) as tc:
        with (
            tc.tile_pool(name="big", bufs=1) as big,
            tc.tile_pool(name="stp", bufs=4) as stp,
            tc.tile_pool(name="gat", bufs=4) as gat,
            tc.tile_pool(name="mid", bufs=4) as mid,
            tc.tile_pool(name="small", bufs=1) as small,
            tc.tile_pool(name="gpsum", bufs=8, space="PSUM") as gpsum,
        ):
            xc = big.tile([P, 4, 2, 2 * RB], fp8, tag="xc", name="xc")
            xt = big.tile([P, 4, 2, (NCHL - 4) * P], fp8, tag="xt", name="xt")
            nbc = small.tile([P, NCHL], f32, tag="nbc", name="nbc")
            idxt = big.tile([P, NQ, KQ // 16], i16, tag="idx", name="idx")
            selt = big.tile([P, NQ, 4, K], fp8, tag="sel", name="sel")
            narow = small.tile([2, 2, RB], fp8, tag="narow", name="narow")
            ones4 = small.tile([2, 2, P], fp8, tag="ones4", name="ones4")
            dum = small.tile([1, 8], f32, tag="dum", name="dum")

            # Warm the ACT function table that contains Sqrt before the
            # pipeline needs it (saves a mid-stream LoadActFuncSet).
            nc.vector.memset(dum[:], 1.0)
            nc.scalar.activation(dum[:], dum[:], Act.Sqrt)
            nc.vector.memset(ones4[:], 4.0)

            # DMA order: serialized stream; lead with quad 0's operands so
            # PE warms up immediately, trail sel slices (needed latest).
            for t in range(4):
                nc.sync.dma_start(xc[:, t], xc_d[:, t])
            nc.sync.dma_start(narow[:], narow_d[:])
            nc.sync.dma_start(nbc[:], nbc_d[:])
            nc.sync.dma_start(idxt[:], idx_d[:])
            for blk in range(1, NBLK):
                cols = slice(512 * (blk - 1), 512 * blk)
                nc.sync.dma_start(xt[:, :, :, cols], xt_d[:, :, :, cols])
                if blk >= 2:
                    nc.sync.dma_start(selt[:, blk - 2], sel_d[:, blk - 2])
            nc.sync.dma_start(selt[:, 3], sel_d[:, 3])
            nc.sync.dma_start(selt[:, 4], sel_d[:, 4])

            acct = small.tile([P, 4 * NACC], f32, tag="acct", name="acct")

            def do_chunk(s4, ch, j):
                g = gpsum.tile([P, RB], f32, tag="g", name="g", space="PSUM")
                for t in range(4):
                    if ch < 4:
                        lhsT = xc[:, t, :, RB + ch * P : RB + (ch + 1) * P]
                    else:
                        lhsT = xt[:, t, :, (ch - 4) * P : (ch - 3) * P]
                    nc.tensor.matmul(
                        g[:],
                        lhsT,
                        xc[:, t, :, 0:RB],
                        start=(t == 0),
                        stop=False,
                        perf_mode=DR,
                    )
                # fold -(na+SQB)/2 via fp8 hi/lo rows scaled by 4.0
                nc.tensor.matmul(
                    g[:], ones4[:], narow[:],
                    start=False, stop=True, perf_mode=DR,
                )
                # strip: s = -2*G + na + nb + SQB = d^2 + SQB
                # (ACT only: DVE tensor_scalar drains showed a rare
                # nondeterministic race against the Pool gather)
                nc.scalar.activation(
                    s4[:, j], g[:], Act.Sqrt,
                    bias=nbc[:, ch : ch + 1], scale=-2.0,
                )

            def chain(q, gth, c0, c1, acol):
                # masks + accumulation over chunks c0:c1 (strip already = d)
                nck = c1 - c0
                dg = gth[:, c0:c1]
                Am = mid.tile([P, nck, K], fp16, tag="Am", name="Am")
                jk = mid.tile([P, nck, K], fp16, tag="jk", name="jk")
                sl = selt[:, q, c0:c1]
                nc.vector.scalar_tensor_tensor(
                    Am[:, :, 0:KS], dg[:, :, 0:KS], cpos, sl[:, :, 0:KS],
                    Alu.is_gt, Alu.mult,
                    accum_out=acct[:, 0 * NACC + acol : 0 * NACC + acol + 1],
                )
                nc.vector.scalar_tensor_tensor(
                    jk[:, :, 0:KS], dg[:, :, 0:KS], 1.0, Am[:, :, 0:KS],
                    Alu.mult, Alu.mult,
                    accum_out=acct[:, 1 * NACC + acol : 1 * NACC + acol + 1],
                )
                nc.vector.scalar_tensor_tensor(
                    Am[:, :, KS:K], dg[:, :, KS:K], cneg, sl[:, :, KS:K],
                    Alu.is_lt, Alu.mult,
                    accum_out=acct[:, 2 * NACC + acol : 2 * NACC + acol + 1],
                )
                nc.vector.scalar_tensor_tensor(
                    jk[:, :, KS:K], dg[:, :, KS:K], 1.0, Am[:, :, KS:K],
                    Alu.mult, Alu.mult,
                    accum_out=acct[:, 3 * NACC + acol : 3 * NACC + acol + 1],
                )

            jobs = []

            def flush(keep):
                while len(jobs) > keep:
                    chain(*jobs.pop(0))

            acol_half = {0: (0, NQ), NQ - 1: (NQ - 1, NQ + 1)}
            for q in range(NQ):
                s4 = stp.tile([P, 4, RB], f32, tag="s4", name="s4")
                if q in _HALF_QUADS:
                    ca, cb = acol_half[q]
                    gth = gat.tile([P, 4, K], f32, tag="gth", name="gth")
                    for j in range(2):
                        do_chunk(s4, 4 * q + j, j)
                    flush(1)
                    nc.gpsimd.ap_gather(
                        gth[:, 0:2], s4[:, 0:2], idxt[:, q, 0 : KQ // 32],
                        channels=P, num_elems=2 * RB, d=1, num_idxs=2 * K,
                    )
                    jobs.append((q, gth, 0, 2, ca))
                    for j in range(2, 4):
                        do_chunk(s4, 4 * q + j, j)
                    flush(1 if q == 0 else 0)
                    nc.gpsimd.ap_gather(
                        gth[:, 2:4], s4[:, 2:4],
                        idxt[:, q, KQ // 32 : KQ // 16],
                        channels=P, num_elems=2 * RB, d=1, num_idxs=2 * K,
                    )
                    jobs.append((q, gth, 2, 4, cb))
                else:
                    for j in range(4):
                        do_chunk(s4, 4 * q + j, j)
                    gth = gat.tile([P, 4, K], f32, tag="gth", name="gth")
                    flush(1)
                    nc.gpsimd.ap_gather(
                        gth[:], s4[:], idxt[:, q],
                        channels=P, num_elems=4 * RB, d=1, num_idxs=KQ,
                    )
                    jobs.append((q, gth, 0, 4, q))
            flush(0)

            nc.sync.dma_start(out_d[:], acct[:])

    nc.compile()
    return nc


def _cumcount(key):
    """Per-element occurrence index within its key group (vectorized)."""
    n = len(key)
    if n == 0:
        return np.zeros(0, dtype=np.int64)
    order = np.argsort(key, kind="stable")
    sk = key[order]
    start = np.r_[0, np.flatnonzero(sk[1:] != sk[:-1]) + 1]
    lens = np.diff(np.r_[start, n])
    pos = np.arange(n) - np.repeat(start, lens)
    out = np.empty(n, dtype=np.int64)
    out[order] = pos
    return out


def _prep_inputs(batch, labels, triplets, beta):
    import ml_dtypes

    fp8 = ml_dtypes.float8_e4m3

    bv = np.asarray(beta, dtype=np.float64)
    assert np.allclose(bv, bv[0]), "kernel assumes per-class beta is constant"

    trip = np.asarray(triplets).astype(np.int64)
    xq = np.clip(np.asarray(batch, dtype=np.float32), -240.0, 240.0).astype(fp8)
    xqf = xq.astype(np.float32)

    xt_glob = np.ascontiguousarray(
        xqf.T.reshape(4, 2, P, B).transpose(2, 0, 1, 3)
    ).astype(fp8)                                       # [P, 4, 2, B]
    na = (xqf.astype(np.float64) ** 2).sum(1)          # [B] row norms, f64
    na2 = na.reshape(NCORES * 4, P)                    # per global chunk

    # symmetric assignment: pair (a, b) computed by owner(a) if
    # (owner(b) - owner(a)) mod 8 <= 4, else by owner(b); strips at the
    # computing core cover blocks i..i+4 (local chunk = position there)
    comp_l, ch_l, part_l, grp_l, col_l, slot_l, side_l = [], [], [], [], [], [], []
    for side, col in ((0, 1), (1, 2)):
        a = trip[:, 0]
        b = trip[:, col]
        i = a >> 9
        j = b >> 9
        comp = np.where((j - i) % 8 <= 4, i, j)
        row = np.where(comp == i, b, a)
        colm = np.where(comp == i, a, b)
        chg = row >> 7
        ch = 4 * (((chg >> 2) - comp) % 8) + (chg & 3)   # local chunk 0..19
        part = row & 127
        grp = part >> 4
        k = _cumcount(((comp * NCHL + ch) * 8 + grp))
        kmax = int(k.max()) if len(k) else 0
        assert kmax < KS, f"slot overflow side {side}: {kmax}"
        comp_l.append(comp); ch_l.append(ch); part_l.append(part)
        grp_l.append(grp); col_l.append(colm - 512 * comp)
        slot_l.append(side * KS + k); side_l.append(side)

    in_maps = []
    for c in range(NCORES):
        lo = c * RB
        xa = xq[lo : lo + RB]
        xat_host = np.ascontiguousarray(
            xa.astype(np.float32).T.reshape(4, 2, P, RB).transpose(2, 0, 1, 3)
        ).astype(fp8)

        gchunks = [4 * ((c + q) % 8) + r for q in range(NBLK) for r in range(4)]
        xt_full = np.concatenate(
            [xt_glob[:, :, :, 128 * g : 128 * (g + 1)] for g in gchunks], axis=3
        )
        xc_host = np.ascontiguousarray(
            np.concatenate([xat_host, xt_full[:, :, :, 0:512]], axis=3)
        )
        xt_host = np.ascontiguousarray(xt_full[:, :, :, 512:])
        nbc_host = np.ascontiguousarray(na2[gchunks].T.astype(np.float32))

        v = -(na[lo : lo + RB] + SQB) / 8.0     # ones are 4.0; fold scale -2
        narow_host = np.zeros((2, 2, RB), dtype=fp8)
        r = v
        for rowi in range(4):
            q8 = r.astype(fp8)
            narow_host[rowi // 2, rowi % 2] = q8
            r = r - q8.astype(np.float64)

        idx = np.zeros((P, NQ, KQ // 16), dtype=np.int16)
        sel = np.zeros((P, NQ, 4, K), dtype=fp8)
        for comp, ch, part, grp, colm, slot in zip(
            comp_l, ch_l, part_l, grp_l, col_l, slot_l
        ):
            m = comp == c
            chm = ch[m]; partm = part[m]; grpm = grp[m]
            colmm = colm[m]; slotm = slot[m]
            quad = chm >> 2
            halfq = (quad == 0) | (quad == NQ - 1)
            # half quads: two 2-chunk gathers; mids: one 4-chunk gather
            i_ = np.where(halfq, K * (chm & 1) + slotm,
                          K * (chm & 3) + slotm)
            val = np.where(halfq, colmm + 512 * (chm & 1),
                           colmm + 512 * (chm & 3)).astype(np.int16)
            colbase = np.where(halfq & ((chm & 2) == 2), KQ // 32, 0)
            idx[16 * grpm + (i_ % 16), quad, colbase + i_ // 16] = val
            sel[partm, quad, chm & 3, slotm] = 1.0

        in_maps.append(
            {
                "xc": xc_host,
                "xt": xt_host,
                "nbc": nbc_host,
                "narow": narow_host,
                "idx": idx,
                "sel": sel,
            }
        )
    return in_maps


def kernel(batch, labels, triplets, beta):
    global _COMPILED, _THRESH, LAST_RESULTS
    from concourse.bass_utils import run_bass_kernel_spmd

    bv = np.asarray(beta, dtype=np.float64)
    cpos = float(bv[0]) - MARGIN
    cneg = float(bv[0]) + MARGIN
    if _COMPILED is None or _THRESH != (cpos, cneg):
        _COMPILED = _build_nc(cpos, cneg)
        _THRESH = (cpos, cneg)
    nc = _COMPILED

    in_maps = _prep_inputs(batch, labels, triplets, beta)
    trace = bool(int(os.environ.get("KERNEL_TRACE", "0")))
    res = run_bass_kernel_spmd(
        nc, in_maps, core_ids=list(range(NCORES)), trace=trace
    )
    LAST_RESULTS = res

    cntP = sumP = cntN = sumN = 0.0
    for r in res.results:
        o = r["out"].astype(np.float64).reshape(P, 4, NACC)
        cntP += o[:, 0].sum()
        sumP += o[:, 1].sum()
        cntN += o[:, 2].sum()
        sumN += o[:, 3].sum()
    pos = sumP - cpos * cntP
    if cntP > 0:
        pos -= SQB * cntP / (2.0 * (sumP / cntP))  # SQB Taylor correction
    neg = cneg * cntN - sumN
    total = pos + neg
    cnt = cntP + cntN
    loss = total if cnt == 0.0 else total / cnt
    return np.float32(loss)
